# revision 6
# baseline (speedup 1.0000x reference)
"""Trainium2 Bass kernel for nn_CrossAttention_47004122087816.

Math (faithful to the reference's "buggy einsum"):
    xn   = LayerNorm(x) * ln_w + ln_b
    q    = (xn @ Wq) * SCALE            [n, E]
    k, v = split(media @ Wkv)           [m, E] each
    sim  = q @ k^T                      [n, m]
    colsum[j] = sum_i softmax(sim, -1)[i, j]
    out  = (colsum[:, None] * v) @ Wout [m, D]

Sharding: pure data-parallel - batch b=8 over 8 NeuronCores, one batch
element per core, no collectives.

Schedule (v2, PE-starvation fix):
 - ALL HBM loads go through ONE SWDGE (gpsimd) queue in strict priority
   order: Wkv[k] -> media0-3 -> Wkv[v] -> media4-7 -> Wq -> ln -> x0-3
   -> media8-11 -> x4-7 -> media12-15 -> x8-11 -> x12-15 -> Wout, so the
   critical kv path gets ~full HBM bandwidth instead of a fair share.
 - sync (SP HWDGE) queue carries only the 32 DMA-xbar transposes.
 - scalar (ACT HWDGE) queue carries the q0/colsum DRAM-bounce transposes
   and the output stores.
 - ~48 junk matmuls on a zeros tile at t=0 warm the PE HAM clock gate
   (cold PE runs at 1.2 GHz; ~3.4us of activity unlocks 2.4 GHz).
 - PE order: junk, kv0, q0, kv1, q_chunk0, kv2, qc1, kv3, qc2, qc3,
   sim/colsum x16, final.
 - exp uses activation(accum_out=) to fuse the softmax row-sum (frees
   the DVE from 16 x 2.3us TENSOR_REDUCEs).
 - colsum accumulates into ONE psum bank at partitions {0,32,64,96}
   (tile_position col packing), freeing 3 banks for matmul rotation.
"""

import sys

for _p in ("/opt/trn_rl_repo",):
    if _p not in sys.path:
        sys.path.insert(0, _p)

import numpy as np

import concourse.bass as bass  # noqa: F401
import concourse.tile as tile
from concourse import bacc, mybir
from concourse.bass_utils import run_bass_kernel_spmd

B = 8
N = 2048          # x rows per batch element
M = 2048          # media rows per batch element
D = 1024          # model dim
E = 512           # inner dim
P = 128           # partitions
F = 512           # matmul free-dim chunk (one PSUM bank of fp32)
CT = D // P       # 8  c-tiles (contraction over model dim)
ET = E // P       # 4  e-tiles (contraction over inner dim)
NT = N // P       # 16 row tiles
JC = M // F       # 4  column chunks of 512
SCALE = 64 ** -0.5
EPS = 1e-5
N_JUNK = 48       # PE warm-up matmuls

FP = mybir.dt.float32
BF = mybir.dt.bfloat16

AF = mybir.ActivationFunctionType
ALU = mybir.AluOpType
AX = mybir.AxisListType


def _build():
    nc = bacc.Bacc("TRN2", target_bir_lowering=False, debug=False, num_devices=B)

    x = nc.dram_tensor("x", [N, D], FP, kind="ExternalInput").ap()
    media = nc.dram_tensor("media", [M, D], FP, kind="ExternalInput").ap()
    ln_w = nc.dram_tensor("ln_w", [D], FP, kind="ExternalInput").ap()
    ln_b = nc.dram_tensor("ln_b", [D], FP, kind="ExternalInput").ap()
    Wq = nc.dram_tensor("Wq", [D, E], FP, kind="ExternalInput").ap()
    Wkv = nc.dram_tensor("Wkv", [D, 2 * E], FP, kind="ExternalInput").ap()
    Wout = nc.dram_tensor("Wout", [E, D], FP, kind="ExternalInput").ap()
    out = nc.dram_tensor("out", [M, D], FP, kind="ExternalOutput").ap()

    with tile.TileContext(nc) as tc:
        from contextlib import ExitStack

        with ExitStack() as ctx:
            consts = ctx.enter_context(tc.tile_pool(name="consts", bufs=1))
            acts = ctx.enter_context(tc.tile_pool(name="acts", bufs=1))
            xstage = ctx.enter_context(tc.tile_pool(name="xstage", bufs=6))
            mstage = ctx.enter_context(tc.tile_pool(name="mstage", bufs=6))
            xw = ctx.enter_context(tc.tile_pool(name="xw", bufs=2))
            mtw = ctx.enter_context(tc.tile_pool(name="mtw", bufs=2))
            expp = ctx.enter_context(tc.tile_pool(name="expp", bufs=2))
            small = ctx.enter_context(tc.tile_pool(name="small", bufs=6))
            outst = ctx.enter_context(tc.tile_pool(name="outst", bufs=3))
            psum_tr = ctx.enter_context(
                tc.tile_pool(name="psum_tr", bufs=1, space="PSUM")
            )
            psum_mm = ctx.enter_context(
                tc.tile_pool(name="psum_mm", bufs=6, space="PSUM")
            )
            psum_cs = ctx.enter_context(
                tc.tile_pool(name="psum_cs", bufs=1, space="PSUM")
            )
            dram = ctx.enter_context(tc.tile_pool(name="dram", bufs=1, space="DRAM"))

            # ---------------- PE warm-up (HAM clock gate) ----------------
            zb = consts.tile([P, F], BF)
            nc.vector.memset(zb[:], 0.0)
            trash = psum_tr.tile([P, F], FP)
            for _ in range(N_JUNK):
                nc.tensor.matmul(
                    trash[:], lhsT=zb[:, 0:P], rhs=zb[:], start=True, stop=True,
                    skip_group_check=True,
                )

            eps_t = consts.tile([P, 1], FP)
            nc.vector.memset(eps_t[:], EPS)

            # ---------------- weights & activations tiles ----------------
            wkv_b = consts.tile([P, CT, 2 * E], BF)
            wq_b = consts.tile([P, CT, E], BF)
            wout_b = consts.tile([P, ET, D], BF)
            kT = acts.tile([P, ET, M], BF)
            vT = acts.tile([P, ET, M], BF)
            qT = acts.tile([P, ET, N], BF)

            # SWDGE load 1: k-half of Wkv (critical path for first matmuls)
            nc.gpsimd.dma_start(
                wkv_b[:, :, 0:E],
                Wkv[:, 0:E].rearrange("(kt p) e -> p kt e", p=P),
            )

            def media_block(blk, mtw_c):
                msb = mstage.tile([P, D], BF, tag="msb", name=f"msb{blk}")
                nc.gpsimd.dma_start(msb[:], media[blk * P : (blk + 1) * P, :])
                b = blk % 4
                nc.sync.dma_start_transpose(mtw_c[:, :, b * P : (b + 1) * P], msb[:])

            mtw_c0 = mtw.tile([P, CT, F], BF, tag="mtw", name="mtw0")
            for b in range(4):
                media_block(b, mtw_c0)

            # SWDGE: v-half of Wkv, then media chunk 1
            nc.gpsimd.dma_start(
                wkv_b[:, :, E : 2 * E],
                Wkv[:, E : 2 * E].rearrange("(kt p) e -> p kt e", p=P),
            )
            mtw_c1 = mtw.tile([P, CT, F], BF, tag="mtw", name="mtw1")
            for b in range(4, 8):
                media_block(b, mtw_c1)

            # SWDGE: Wq (cast straight to bf16), ln vectors
            nc.gpsimd.dma_start(wq_b[:], Wq.rearrange("(kt p) d -> p kt d", p=P))
            lnw = consts.tile([P, CT], FP)
            lnb_f = consts.tile([P, CT], FP)
            nc.gpsimd.dma_start(lnw[:], ln_w.rearrange("(t p) -> p t", p=P))
            nc.gpsimd.dma_start(lnb_f[:], ln_b.rearrange("(t p) -> p t", p=P))
            lnw_s = consts.tile([P, CT], FP)
            nc.vector.tensor_scalar_mul(lnw_s[:], lnw[:], SCALE)
            lnb_s = consts.tile([P, CT], BF)  # ln_b * SCALE, lhsT for q0
            nc.vector.tensor_scalar_mul(lnb_s[:], lnb_f[:], SCALE)

            def x_block(blk, xw_c):
                # bf16 cast-load; LayerNorm entirely in bf16
                xt = xstage.tile([P, D], BF, tag="xt", name=f"xt{blk}")
                nc.gpsimd.dma_start(xt[:], x[blk * P : (blk + 1) * P, :])
                st = small.tile([P, 2, 6], FP, tag="st", name=f"st{blk}")
                for sg in range(2):
                    nc.vector.bn_stats(st[:, sg, :], xt[:, sg * 512 : (sg + 1) * 512])
                mv = small.tile([P, 2], FP, tag="mv", name=f"mv{blk}")
                nc.vector.bn_aggr(mv[:], st[:])
                sd = small.tile([P, 1], FP, tag="sd", name=f"sd{blk}")
                nc.scalar.activation(
                    sd[:], mv[:, 1:2], func=AF.Sqrt, bias=eps_t[:], scale=1.0
                )
                rsig = small.tile([P, 1], FP, tag="rsig", name=f"rsig{blk}")
                nc.vector.reciprocal(rsig[:], sd[:])
                nmr = small.tile([P, 1], FP, tag="nmr", name=f"nmr{blk}")
                nc.vector.tensor_scalar(
                    nmr[:], mv[:, 0:1], rsig[:], -1.0, ALU.mult, ALU.mult
                )
                xh = xstage.tile([P, D], BF, tag="xh", name=f"xh{blk}")
                nc.scalar.activation(
                    xh[:], xt[:], func=AF.Identity, bias=nmr[:], scale=rsig[:]
                )
                b = blk % 4
                nc.sync.dma_start_transpose(xw_c[:, :, b * P : (b + 1) * P], xh[:])

            xw_c0 = xw.tile([P, CT, F], BF, tag="xw", name="xw0")
            for b in range(4):
                x_block(b, xw_c0)

            def kvT_chunk(jc, mtw_c):
                for et in range(2 * ET):
                    ps = psum_mm.tile([P, F], FP, tag="ps", name=f"kv{jc}_{et}")
                    for kt in range(CT):
                        nc.tensor.matmul(
                            ps[:],
                            lhsT=wkv_b[:, kt, et * P : (et + 1) * P],
                            rhs=mtw_c[:, kt, :],
                            start=(kt == 0),
                            stop=(kt == CT - 1),
                        )
                    if et < ET:
                        nc.scalar.copy(kT[:, et, jc * F : (jc + 1) * F], ps[:])
                    else:
                        nc.vector.tensor_copy(
                            vT[:, et - ET, jc * F : (jc + 1) * F], ps[:]
                        )

            # PE: kv chunk 0 (first real matmul work)
            kvT_chunk(0, mtw_c0)

            # q0 = (SCALE * ln_b) @ Wq  (row bias for q; uses unscaled wq_b)
            q0_ps = psum_cs.tile([1, E], FP, tag="cs", name="q0ps")
            for kt in range(CT):
                nc.tensor.matmul(
                    q0_ps[:],
                    lhsT=lnb_s[:, kt : kt + 1],
                    rhs=wq_b[:, kt, :],
                    start=(kt == 0),
                    stop=(kt == CT - 1),
                )
            q0_sb = consts.tile([1, E], FP)
            nc.scalar.copy(q0_sb[:], q0_ps[:])
            # DRAM bounce transpose: q0 [1, E] -> q0T [P, ET]
            q0_dr = dram.tile([E], FP)
            nc.scalar.dma_start(q0_dr[:], q0_sb[0:1, :])
            q0T = consts.tile([P, ET], FP)
            nc.scalar.dma_start(q0T[:], q0_dr.rearrange("(t p) -> p t", p=P))

            # in-place: wq_b <- (SCALE * ln_w) (x)_rows Wq  (after q0 reads it)
            for kt in range(CT):
                nc.scalar.mul(wq_b[:, kt], wq_b[:, kt], lnw_s[:, kt : kt + 1])

            # SWDGE: media chunk 2 loads, then x chunk 1 loads
            mtw_c2 = mtw.tile([P, CT, F], BF, tag="mtw", name="mtw2")
            for b in range(8, 12):
                media_block(b, mtw_c2)

            kvT_chunk(1, mtw_c1)

            xw_c1 = xw.tile([P, CT, F], BF, tag="xw", name="xw1")
            for b in range(4, 8):
                x_block(b, xw_c1)

            def qT_chunk(ic, xw_c):
                for dt in range(ET):
                    ps = psum_mm.tile([P, F], FP, tag="ps", name=f"q{ic}_{dt}")
                    for kt in range(CT):
                        nc.tensor.matmul(
                            ps[:],
                            lhsT=wq_b[:, kt, dt * P : (dt + 1) * P],
                            rhs=xw_c[:, kt, :],
                            start=(kt == 0),
                            stop=(kt == CT - 1),
                        )
                    nc.vector.tensor_scalar_add(
                        qT[:, dt, ic * F : (ic + 1) * F], ps[:], q0T[:, dt : dt + 1]
                    )

            qT_chunk(0, xw_c0)

            mtw_c3 = mtw.tile([P, CT, F], BF, tag="mtw", name="mtw3")
            for b in range(12, 16):
                media_block(b, mtw_c3)

            kvT_chunk(2, mtw_c2)

            xw_c2 = xw.tile([P, CT, F], BF, tag="xw", name="xw2")
            for b in range(8, 12):
                x_block(b, xw_c2)

            qT_chunk(1, xw_c1)

            kvT_chunk(3, mtw_c3)

            xw_c3 = xw.tile([P, CT, F], BF, tag="xw", name="xw3")
            for b in range(12, 16):
                x_block(b, xw_c3)

            qT_chunk(2, xw_c2)
            qT_chunk(3, xw_c3)

            # SWDGE tail: Wout cast-load (phase 3 weight)
            nc.gpsimd.dma_start(
                wout_b[:], Wout.rearrange("(et p) d -> p et d", p=P)
            )

            # ---------------- sim, exp(+rowsum), colsum ----------------
            # colsum accumulates in ONE psum bank: chunk jc at partition 32*jc
            cs = psum_cs.tile([P, F], FP, tag="cs", name="csbank")
            exs: list = [None, None]  # software pipeline: colsum lags sim by 1
            zrbs: list = [None, None]

            def colsum_mms(it):
                ex_p, zrb_p = exs[it % 2], zrbs[it % 2]
                for jc in range(JC):
                    nc.tensor.matmul(
                        cs[32 * jc : 32 * jc + 1, :],
                        lhsT=zrb_p[:],
                        rhs=ex_p[:, jc * F : (jc + 1) * F],
                        start=(it == 0),
                        stop=(it == NT - 1),
                        skip_group_check=True,
                        tile_position=(0, 32 * jc),
                    )

            for it in range(NT):
                ex = expp.tile([P, M], BF, tag="ex", name=f"ex{it}")
                zp = small.tile([P, JC], FP, tag="zp", name=f"zp{it}")
                for jc in range(JC):
                    ps = psum_mm.tile([P, F], FP, tag="ps", name=f"sim{it}_{jc}")
                    for et in range(ET):
                        nc.tensor.matmul(
                            ps[:],
                            lhsT=qT[:, et, it * P : (it + 1) * P],
                            rhs=kT[:, et, jc * F : (jc + 1) * F],
                            start=(et == 0),
                            stop=(et == ET - 1),
                        )
                    nc.scalar.activation(
                        ex[:, jc * F : (jc + 1) * F], ps[:], func=AF.Exp,
                        accum_out=zp[:, jc : jc + 1],
                    )
                z = small.tile([P, 1], FP, tag="z", name=f"z{it}")
                nc.vector.tensor_reduce(z[:], zp[:], axis=AX.X, op=ALU.add)
                zr = small.tile([P, 1], FP, tag="zr", name=f"zr{it}")
                nc.vector.reciprocal(zr[:], z[:])
                zrb = small.tile([P, 1], BF, tag="zrb", name=f"zrb{it}")
                nc.vector.tensor_copy(zrb[:], zr[:])
                exs[it % 2], zrbs[it % 2] = ex, zrb
                if it > 0:
                    colsum_mms(it - 1)

            # ---------------- final: out = (colsum (x) v) @ Wout ----------------
            def final_mms(jt):
                pss = []
                for n2 in range(2):
                    ps = psum_mm.tile([P, F], FP, tag="ps", name=f"y{jt}_{n2}")
                    for et in range(ET):
                        nc.tensor.matmul(
                            ps[:],
                            lhsT=vT[:, et, jt * P : (jt + 1) * P],
                            rhs=wout_b[:, et, n2 * F : (n2 + 1) * F],
                            start=(et == 0),
                            stop=(et == ET - 1),
                        )
                    pss.append(ps)
                return pss

            def final_evac(jt, pss, scol, ot):
                for n2, ps in enumerate(pss):
                    if n2 == 0:
                        nc.scalar.mul(
                            ot[:, n2 * F : (n2 + 1) * F], ps[:], scol[:, jt : jt + 1]
                        )
                    else:
                        nc.vector.tensor_scalar_mul(
                            ot[:, n2 * F : (n2 + 1) * F], ps[:], scol[:, jt : jt + 1]
                        )
                nc.scalar.dma_start(out[jt * P : (jt + 1) * P, :], ot[:])

            # first final j-tiles issue while the last exp/colsum drains,
            # keeping the PE busy through the softmax tail
            colsum_mms(NT - 1)
            early = [final_mms(jt) for jt in range(3)]

            # evacuate colsum bank -> SBUF (same partitions) -> DRAM bounce
            csb4 = consts.tile([P, F], FP)
            for jc in range(JC):
                nc.scalar.copy(
                    csb4[32 * jc : 32 * jc + 1, :], cs[32 * jc : 32 * jc + 1, :]
                )
            cs_dr = dram.tile([M], FP)
            for jc in range(JC):
                nc.scalar.dma_start(
                    cs_dr[jc * F : (jc + 1) * F], csb4[32 * jc : 32 * jc + 1, :]
                )
            scol = consts.tile([P, NT], FP)
            nc.scalar.dma_start(scol[:], cs_dr.rearrange("(t p) -> p t", p=P))

            for jt in range(3):
                ot = outst.tile([P, D], FP, tag="ot", name=f"ot{jt}")
                final_evac(jt, early[jt], scol, ot)
            for jt in range(3, NT):
                pss = final_mms(jt)
                ot = outst.tile([P, D], FP, tag="ot", name=f"ot{jt}")
                final_evac(jt, pss, scol, ot)

    nc.compile()
    return nc


_NC_CACHE = None


def _get_nc():
    global _NC_CACHE
    if _NC_CACHE is None:
        _NC_CACHE = _build()
    return _NC_CACHE


def _run(inputs, trace=False, **kw):
    nc = _get_nc()
    shared = {
        k: np.ascontiguousarray(np.asarray(inputs[k], dtype=np.float32))
        for k in ("ln_w", "ln_b", "Wq", "Wkv", "Wout")
    }
    xs = np.ascontiguousarray(np.asarray(inputs["x"], dtype=np.float32))
    ms = np.ascontiguousarray(np.asarray(inputs["media"], dtype=np.float32))
    in_maps = [dict(shared, x=xs[b], media=ms[b]) for b in range(B)]
    res = run_bass_kernel_spmd(nc, in_maps, core_ids=list(range(B)), trace=trace, **kw)
    out = np.stack([res.results[b]["out"] for b in range(B)], axis=0)
    return out, res


def kernel(**inputs) -> np.ndarray:
    out, _ = _run(inputs, trace=False)
    return out


# revision 7
# speedup vs baseline: 1.2134x; 1.2134x over previous
"""Trainium2 Bass kernel for nn_CrossAttention_47004122087816.

Math (faithful to the reference's "buggy einsum"):
    xn   = LayerNorm(x) * ln_w + ln_b
    q    = (xn @ Wq) * SCALE            [n, E]
    k, v = split(media @ Wkv)           [m, E] each
    sim  = q @ k^T                      [n, m]
    colsum[j] = sum_i softmax(sim, -1)[i, j]
    out  = (colsum[:, None] * v) @ Wout [m, D]

Sharding: pure data-parallel - batch b=8 over 8 NeuronCores, one batch
element per core, no collectives.

Schedule (v3): the kernel is PE-bound (~180us of matmul work); the
feed must never stall the PE stream.
 - ALL HBM loads go through ONE SWDGE (gpsimd) queue, each a ~2MB-read
   chunk (4 row-tiles per DMA), ordered by PE need-time:
   Wq, ln, x0, m0, Wkv[k], Wkv[v], m1, x1, m2, x2, m3, x3, Wout.
   A single queue gives the head-of-line ~full HBM bandwidth; chunked
   transfers keep the ~5-deep in-flight window covering multiple MB.
 - PE order: junk-warmup, q0, q_chunk0, junk2, kv0, kv1, qc1, kv2,
   qc2, kv3, sim x16 (qc3 slotted before it=8), final.  Junk matmuls
   on a zeros tile keep the HAM clock gate warm (cold PE = 1.2 GHz).
 - sync (SP HWDGE) queue carries only the 32 DMA-xbar transposes.
 - scalar (ACT HWDGE) queue carries the q0/colsum DRAM-bounce
   transposes and the output stores.
 - exp uses activation(accum_out=) to fuse the softmax row-sum.
 - colsum accumulates into ONE psum bank at partitions {0,32,64,96}
   (tile_position col packing), freeing 3 banks for matmul rotation.
"""

import sys

for _p in ("/opt/trn_rl_repo",):
    if _p not in sys.path:
        sys.path.insert(0, _p)

import numpy as np

import concourse.bass as bass  # noqa: F401
import concourse.tile as tile
from concourse import bacc, mybir
from concourse.bass_utils import run_bass_kernel_spmd

B = 8
N = 2048          # x rows per batch element
M = 2048          # media rows per batch element
D = 1024          # model dim
E = 512           # inner dim
P = 128           # partitions
F = 512           # matmul free-dim chunk (one PSUM bank of fp32)
CT = D // P       # 8  c-tiles (contraction over model dim)
ET = E // P       # 4  e-tiles (contraction over inner dim)
NT = N // P       # 16 row tiles
JC = M // F       # 4  column chunks of 512
SCALE = 64 ** -0.5
EPS = 1e-5
N_JUNK = 24       # PE warm-up matmuls at kernel start
N_JUNK2 = 16      # PE filler matmuls while Wkv streams in

FP = mybir.dt.float32
BF = mybir.dt.bfloat16

AF = mybir.ActivationFunctionType
ALU = mybir.AluOpType
AX = mybir.AxisListType


def _build():
    nc = bacc.Bacc("TRN2", target_bir_lowering=False, debug=False, num_devices=B)

    x = nc.dram_tensor("x", [N, D], FP, kind="ExternalInput").ap()
    media = nc.dram_tensor("media", [M, D], FP, kind="ExternalInput").ap()
    ln_w = nc.dram_tensor("ln_w", [D], FP, kind="ExternalInput").ap()
    ln_b = nc.dram_tensor("ln_b", [D], FP, kind="ExternalInput").ap()
    Wq = nc.dram_tensor("Wq", [D, E], FP, kind="ExternalInput").ap()
    Wkv = nc.dram_tensor("Wkv", [D, 2 * E], FP, kind="ExternalInput").ap()
    Wout = nc.dram_tensor("Wout", [E, D], FP, kind="ExternalInput").ap()
    out = nc.dram_tensor("out", [M, D], FP, kind="ExternalOutput").ap()

    with tile.TileContext(nc) as tc:
        from contextlib import ExitStack

        with ExitStack() as ctx:
            consts = ctx.enter_context(tc.tile_pool(name="consts", bufs=1))
            acts = ctx.enter_context(tc.tile_pool(name="acts", bufs=1))
            xstage = ctx.enter_context(tc.tile_pool(name="xstage", bufs=2))
            xhst = ctx.enter_context(tc.tile_pool(name="xhst", bufs=6))
            mstage = ctx.enter_context(tc.tile_pool(name="mstage", bufs=2))
            xw = ctx.enter_context(tc.tile_pool(name="xw", bufs=2))
            mtw = ctx.enter_context(tc.tile_pool(name="mtw", bufs=2))
            expp = ctx.enter_context(tc.tile_pool(name="expp", bufs=2))
            small = ctx.enter_context(tc.tile_pool(name="small", bufs=6))
            outst = ctx.enter_context(tc.tile_pool(name="outst", bufs=3))
            psum_tr = ctx.enter_context(
                tc.tile_pool(name="psum_tr", bufs=1, space="PSUM")
            )
            psum_mm = ctx.enter_context(
                tc.tile_pool(name="psum_mm", bufs=6, space="PSUM")
            )
            psum_cs = ctx.enter_context(
                tc.tile_pool(name="psum_cs", bufs=1, space="PSUM")
            )
            dram = ctx.enter_context(tc.tile_pool(name="dram", bufs=1, space="DRAM"))

            # ---------------- PE warm-up (HAM clock gate) ----------------
            zb = consts.tile([P, F], BF)
            nc.vector.memset(zb[:], 0.0)
            trash = psum_tr.tile([P, F], FP)

            def junk_mms(n):
                for _ in range(n):
                    nc.tensor.matmul(
                        trash[:], lhsT=zb[:, 0:P], rhs=zb[:], start=True, stop=True,
                        skip_group_check=True,
                    )

            junk_mms(N_JUNK)

            eps_t = consts.tile([P, 1], FP)
            nc.vector.memset(eps_t[:], EPS)

            # ---------------- tiles ----------------
            wkv_b = consts.tile([P, CT, 2 * E], BF)
            wq_b = consts.tile([P, CT, E], BF)
            wout_b = consts.tile([P, ET, D], BF)
            kT = acts.tile([P, ET, M], BF)
            vT = acts.tile([P, ET, M], BF)
            qT = acts.tile([P, ET, N], BF)

            # ---------------- SWDGE queue head: Wq, ln, x0, m0 ----------------
            nc.gpsimd.dma_start(wq_b[:], Wq.rearrange("(kt p) d -> p kt d", p=P))
            lnw = consts.tile([P, CT], FP)
            lnb_f = consts.tile([P, CT], FP)
            nc.gpsimd.dma_start(lnw[:], ln_w.rearrange("(t p) -> p t", p=P))
            nc.gpsimd.dma_start(lnb_f[:], ln_b.rearrange("(t p) -> p t", p=P))
            lnw_s = consts.tile([P, CT], FP)
            nc.vector.tensor_scalar_mul(lnw_s[:], lnw[:], SCALE)
            lnb_s = consts.tile([P, CT], BF)  # ln_b * SCALE, lhsT for q0
            nc.vector.tensor_scalar_mul(lnb_s[:], lnb_f[:], SCALE)

            def x_chunk_load(c):
                # one 2MB-read cast DMA for 4 row tiles
                xt_c = xstage.tile([P, 4, D], BF, tag="xt", name=f"xt{c}")
                nc.gpsimd.dma_start(
                    xt_c[:],
                    x[c * 4 * P : (c + 1) * 4 * P, :].rearrange(
                        "(b p) d -> p b d", p=P
                    ),
                )
                return xt_c

            def x_chunk_ln(c, xt_c, xw_c):
                for b in range(4):
                    xt = xt_c[:, b, :]
                    st = small.tile([P, 2, 6], FP, tag="st", name=f"st{c}_{b}")
                    for sg in range(2):
                        nc.vector.bn_stats(
                            st[:, sg, :], xt[:, sg * 512 : (sg + 1) * 512]
                        )
                    mv = small.tile([P, 2], FP, tag="mv", name=f"mv{c}_{b}")
                    nc.vector.bn_aggr(mv[:], st[:])
                    sd = small.tile([P, 1], FP, tag="sd", name=f"sd{c}_{b}")
                    nc.scalar.activation(
                        sd[:], mv[:, 1:2], func=AF.Sqrt, bias=eps_t[:], scale=1.0
                    )
                    rsig = small.tile([P, 1], FP, tag="rsig", name=f"rsig{c}_{b}")
                    nc.vector.reciprocal(rsig[:], sd[:])
                    nmr = small.tile([P, 1], FP, tag="nmr", name=f"nmr{c}_{b}")
                    nc.vector.tensor_scalar(
                        nmr[:], mv[:, 0:1], rsig[:], -1.0, ALU.mult, ALU.mult
                    )
                    xh = xhst.tile([P, D], BF, tag="xh", name=f"xh{c}_{b}")
                    nc.scalar.activation(
                        xh[:], xt, func=AF.Identity, bias=nmr[:], scale=rsig[:]
                    )
                    nc.sync.dma_start_transpose(
                        xw_c[:, :, b * P : (b + 1) * P], xh[:]
                    )

            def media_chunk(c, mtw_c):
                msb_c = mstage.tile([P, 4, D], BF, tag="msb", name=f"msb{c}")
                nc.gpsimd.dma_start(
                    msb_c[:],
                    media[c * 4 * P : (c + 1) * 4 * P, :].rearrange(
                        "(b p) d -> p b d", p=P
                    ),
                )
                for b in range(4):
                    nc.sync.dma_start_transpose(
                        mtw_c[:, :, b * P : (b + 1) * P], msb_c[:, b, :]
                    )

            # x chunk 0 then media chunk 0 loads
            xw_c0 = xw.tile([P, CT, F], BF, tag="xw", name="xw0")
            xt_c0 = x_chunk_load(0)
            x_chunk_ln(0, xt_c0, xw_c0)
            mtw_c0 = mtw.tile([P, CT, F], BF, tag="mtw", name="mtw0")
            media_chunk(0, mtw_c0)

            # Wkv halves
            nc.gpsimd.dma_start(
                wkv_b[:, :, 0:E],
                Wkv[:, 0:E].rearrange("(kt p) e -> p kt e", p=P),
            )
            nc.gpsimd.dma_start(
                wkv_b[:, :, E : 2 * E],
                Wkv[:, E : 2 * E].rearrange("(kt p) e -> p kt e", p=P),
            )

            # ---------------- PE: q0, q chunk 0 ----------------
            # q0 = (SCALE * ln_b) @ Wq  (row bias for q; uses unscaled wq_b)
            q0_ps = psum_cs.tile([1, E], FP, tag="cs", name="q0ps")
            for kt in range(CT):
                nc.tensor.matmul(
                    q0_ps[:],
                    lhsT=lnb_s[:, kt : kt + 1],
                    rhs=wq_b[:, kt, :],
                    start=(kt == 0),
                    stop=(kt == CT - 1),
                )
            q0_sb = consts.tile([1, E], FP)
            nc.scalar.copy(q0_sb[:], q0_ps[:])
            # DRAM bounce transpose: q0 [1, E] -> q0T [P, ET]
            q0_dr = dram.tile([E], FP)
            nc.scalar.dma_start(q0_dr[:], q0_sb[0:1, :])
            q0T = consts.tile([P, ET], FP)
            nc.scalar.dma_start(q0T[:], q0_dr.rearrange("(t p) -> p t", p=P))

            # in-place: wq_b <- (SCALE * ln_w) (x)_rows Wq  (after q0 reads it)
            for kt in range(CT):
                nc.scalar.mul(wq_b[:, kt], wq_b[:, kt], lnw_s[:, kt : kt + 1])

            def qT_chunk(ic, xw_c):
                for dt in range(ET):
                    ps = psum_mm.tile([P, F], FP, tag="ps", name=f"q{ic}_{dt}")
                    for kt in range(CT):
                        nc.tensor.matmul(
                            ps[:],
                            lhsT=wq_b[:, kt, dt * P : (dt + 1) * P],
                            rhs=xw_c[:, kt, :],
                            start=(kt == 0),
                            stop=(kt == CT - 1),
                        )
                    nc.vector.tensor_scalar_add(
                        qT[:, dt, ic * F : (ic + 1) * F], ps[:], q0T[:, dt : dt + 1]
                    )

            qT_chunk(0, xw_c0)
            junk_mms(N_JUNK2)  # cover the Wkv arrival gap

            def kvT_chunk(jc, mtw_c):
                for et in range(2 * ET):
                    ps = psum_mm.tile([P, F], FP, tag="ps", name=f"kv{jc}_{et}")
                    for kt in range(CT):
                        nc.tensor.matmul(
                            ps[:],
                            lhsT=wkv_b[:, kt, et * P : (et + 1) * P],
                            rhs=mtw_c[:, kt, :],
                            start=(kt == 0),
                            stop=(kt == CT - 1),
                        )
                    if et < ET:
                        nc.scalar.copy(kT[:, et, jc * F : (jc + 1) * F], ps[:])
                    else:
                        nc.vector.tensor_copy(
                            vT[:, et - ET, jc * F : (jc + 1) * F], ps[:]
                        )

            kvT_chunk(0, mtw_c0)

            # SWDGE: m1, x1 loads; PE kv1, qc1
            mtw_c1 = mtw.tile([P, CT, F], BF, tag="mtw", name="mtw1")
            media_chunk(1, mtw_c1)
            xw_c1 = xw.tile([P, CT, F], BF, tag="xw", name="xw1")
            xt_c1 = x_chunk_load(1)
            x_chunk_ln(1, xt_c1, xw_c1)

            kvT_chunk(1, mtw_c1)

            mtw_c2 = mtw.tile([P, CT, F], BF, tag="mtw", name="mtw2")
            media_chunk(2, mtw_c2)

            qT_chunk(1, xw_c1)

            xw_c2 = xw.tile([P, CT, F], BF, tag="xw", name="xw2")
            xt_c2 = x_chunk_load(2)
            x_chunk_ln(2, xt_c2, xw_c2)

            kvT_chunk(2, mtw_c2)

            mtw_c3 = mtw.tile([P, CT, F], BF, tag="mtw", name="mtw3")
            media_chunk(3, mtw_c3)

            qT_chunk(2, xw_c2)

            xw_c3 = xw.tile([P, CT, F], BF, tag="xw", name="xw3")
            xt_c3 = x_chunk_load(3)
            x_chunk_ln(3, xt_c3, xw_c3)

            kvT_chunk(3, mtw_c3)

            # SWDGE tail: Wout cast-load (phase 3 weight)
            nc.gpsimd.dma_start(
                wout_b[:], Wout.rearrange("(et p) d -> p et d", p=P)
            )

            # ---------------- sim, exp(+rowsum), colsum ----------------
            # colsum accumulates in ONE psum bank: chunk jc at partition 32*jc
            cs = psum_cs.tile([P, F], FP, tag="cs", name="csbank")
            exs: list = [None, None]  # software pipeline: colsum lags sim by 1
            zrbs: list = [None, None]

            def colsum_mms(it):
                ex_p, zrb_p = exs[it % 2], zrbs[it % 2]
                for jc in range(JC):
                    nc.tensor.matmul(
                        cs[32 * jc : 32 * jc + 1, :],
                        lhsT=zrb_p[:],
                        rhs=ex_p[:, jc * F : (jc + 1) * F],
                        start=(it == 0),
                        stop=(it == NT - 1),
                        skip_group_check=True,
                        tile_position=(0, 32 * jc),
                    )

            for it in range(NT):
                if it == 8:
                    qT_chunk(3, xw_c3)
                ex = expp.tile([P, M], BF, tag="ex", name=f"ex{it}")
                zp = small.tile([P, JC], FP, tag="zp", name=f"zp{it}")
                for jc in range(JC):
                    ps = psum_mm.tile([P, F], FP, tag="ps", name=f"sim{it}_{jc}")
                    for et in range(ET):
                        nc.tensor.matmul(
                            ps[:],
                            lhsT=qT[:, et, it * P : (it + 1) * P],
                            rhs=kT[:, et, jc * F : (jc + 1) * F],
                            start=(et == 0),
                            stop=(et == ET - 1),
                        )
                    nc.scalar.activation(
                        ex[:, jc * F : (jc + 1) * F], ps[:], func=AF.Exp,
                        accum_out=zp[:, jc : jc + 1],
                    )
                z = small.tile([P, 1], FP, tag="z", name=f"z{it}")
                nc.vector.tensor_reduce(z[:], zp[:], axis=AX.X, op=ALU.add)
                zr = small.tile([P, 1], FP, tag="zr", name=f"zr{it}")
                nc.vector.reciprocal(zr[:], z[:])
                zrb = small.tile([P, 1], BF, tag="zrb", name=f"zrb{it}")
                nc.vector.tensor_copy(zrb[:], zr[:])
                exs[it % 2], zrbs[it % 2] = ex, zrb
                if it > 0:
                    colsum_mms(it - 1)

            # ---------------- final: out = (colsum (x) v) @ Wout ----------------
            def final_mms(jt):
                pss = []
                for n2 in range(2):
                    ps = psum_mm.tile([P, F], FP, tag="ps", name=f"y{jt}_{n2}")
                    for et in range(ET):
                        nc.tensor.matmul(
                            ps[:],
                            lhsT=vT[:, et, jt * P : (jt + 1) * P],
                            rhs=wout_b[:, et, n2 * F : (n2 + 1) * F],
                            start=(et == 0),
                            stop=(et == ET - 1),
                        )
                    pss.append(ps)
                return pss

            def final_evac(jt, pss, scol, ot):
                for n2, ps in enumerate(pss):
                    if n2 == 0:
                        nc.scalar.mul(
                            ot[:, n2 * F : (n2 + 1) * F], ps[:], scol[:, jt : jt + 1]
                        )
                    else:
                        nc.vector.tensor_scalar_mul(
                            ot[:, n2 * F : (n2 + 1) * F], ps[:], scol[:, jt : jt + 1]
                        )
                nc.scalar.dma_start(out[jt * P : (jt + 1) * P, :], ot[:])

            # first final j-tiles issue while the last exp/colsum drains,
            # keeping the PE busy through the softmax tail
            colsum_mms(NT - 1)
            early = [final_mms(jt) for jt in range(3)]

            # evacuate colsum bank -> SBUF (same partitions) -> DRAM bounce
            csb4 = consts.tile([P, F], FP)
            for jc in range(JC):
                nc.scalar.copy(
                    csb4[32 * jc : 32 * jc + 1, :], cs[32 * jc : 32 * jc + 1, :]
                )
            cs_dr = dram.tile([M], FP)
            for jc in range(JC):
                nc.scalar.dma_start(
                    cs_dr[jc * F : (jc + 1) * F], csb4[32 * jc : 32 * jc + 1, :]
                )
            scol = consts.tile([P, NT], FP)
            nc.scalar.dma_start(scol[:], cs_dr.rearrange("(t p) -> p t", p=P))

            for jt in range(3):
                ot = outst.tile([P, D], FP, tag="ot", name=f"ot{jt}")
                final_evac(jt, early[jt], scol, ot)
            for jt in range(3, NT):
                pss = final_mms(jt)
                ot = outst.tile([P, D], FP, tag="ot", name=f"ot{jt}")
                final_evac(jt, pss, scol, ot)

    nc.compile()
    return nc


_NC_CACHE = None


def _get_nc():
    global _NC_CACHE
    if _NC_CACHE is None:
        _NC_CACHE = _build()
    return _NC_CACHE


def _run(inputs, trace=False, **kw):
    nc = _get_nc()
    shared = {
        k: np.ascontiguousarray(np.asarray(inputs[k], dtype=np.float32))
        for k in ("ln_w", "ln_b", "Wq", "Wkv", "Wout")
    }
    xs = np.ascontiguousarray(np.asarray(inputs["x"], dtype=np.float32))
    ms = np.ascontiguousarray(np.asarray(inputs["media"], dtype=np.float32))
    in_maps = [dict(shared, x=xs[b], media=ms[b]) for b in range(B)]
    res = run_bass_kernel_spmd(nc, in_maps, core_ids=list(range(B)), trace=trace, **kw)
    out = np.stack([res.results[b]["out"] for b in range(B)], axis=0)
    return out, res


def kernel(**inputs) -> np.ndarray:
    out, _ = _run(inputs, trace=False)
    return out
